# revision 18
# baseline (speedup 1.0000x reference)
"""Cross-attention block on 8 Trainium2 NeuronCores (Bass/Tile).

Reference computation (all f32):
    func_val = func_embed @ Wv_w.T + Wv_b          # [8192, 256]
    z        = (sent_embed @ func_embed.T) / 16    # [16384, 8192]
    out      = softmax(z, axis=1) @ func_val       # [16384, 256]

Sharding: sent_embed rows split across 8 cores (2048 rows each);
func_embed / Wv replicated. Each core runs an identical flash-style
fused kernel:

  * scores are computed TRANSPOSED (zT[k, q]) so that e = exp(zT) is
    already in the [contraction, out-partition] layout the second
    matmul needs as its stationary operand — no on-device transposes.
  * softmax needs no max subtraction (z ~ N(0,1)); the denominator
    falls out of the same matmul via a ones column: U = e.T @ [V | 1],
    out = U[:, :256] * (1 / U[:, 256]).
  * the e.T @ [V|1] matmul runs in fp8-e4m3 DoubleRow perf mode
    (2 contraction k-tiles per instruction = 2x PE throughput).  exp
    output is scaled by 2^-5 (folded into the activation bias) to keep
    e within fp8e4 range; the scale cancels in the U-num/U-den ratio.
  * exp is evaluated over k-tile PAIRS ([128,1024] per instruction,
    PSUM tiles spanning two banks) halving ACT per-instruction
    overhead — ACT is the second-busiest engine.
  * optionally a fraction of score k-tiles also run in fp8 DoubleRow
    (FP8Z_PERIOD); quantization noise there is the error budget's
    dominant term so only a minority of tiles may use it.

Host side only reshapes/transposes/casts (layout prep + sharding);
every FLOP of the model runs on device.
"""

import numpy as np

import concourse.bass as bass  # noqa: F401  (bass types used via tile/bacc)
import concourse.tile as tile
from concourse import bacc, mybir
from concourse.bass_utils import run_bass_kernel_spmd

N_CORES = 8
N_FUNC = 8192
N_SENT = 16384
D = 256
QROWS = N_SENT // N_CORES          # 2048 query rows per core
QB = 512                           # query block per flash iteration
NKT = N_FUNC // 128                # 64 key tiles
NP = NKT // 2                      # 32 key-tile pairs
NV = D + 1                         # V augmented with ones column
NQB = QROWS // QB
NSUB = QB // 128

# every FP8Z_PERIOD-th key-tile pair computes its scores in fp8
# DoubleRow too (0 = never). 2 -> ~half of scores in fp8.
FP8Z_PERIOD = 2
# first E0 pairs of block 0 are issued during the V' pre-phase (always
# bf16 there: ft8 hasn't landed yet and it keeps the weave simple)
E0 = 10

BF = mybir.dt.bfloat16
F8 = mybir.dt.float8e4
BF_NP = mybir.dt.np(BF)
F8_NP = mybir.dt.np(F8)

EPRE_LOG2 = -5.0                   # e is scaled by 2^-5 before fp8 cast
EBIAS = EPRE_LOG2 * 0.6931471805599453

_COMPILED = None


def _fp8z(b, p):
    if FP8Z_PERIOD == 0 or (b == 0 and p < E0):
        return False
    return p % FP8Z_PERIOD == FP8Z_PERIOD - 1


def build():
    nc = bacc.Bacc("TRN2", target_bir_lowering=False, debug=False,
                   num_devices=N_CORES)
    f32 = mybir.dt.float32
    ft = nc.dram_tensor("ft", [128, 2, N_FUNC], BF, kind="ExternalInput").ap()
    qt = nc.dram_tensor("qt", [128, 2, QROWS], BF, kind="ExternalInput").ap()
    wvt = nc.dram_tensor("wvt", [128, 2, D], BF, kind="ExternalInput").ap()
    bias = nc.dram_tensor("bias", [128, D], f32, kind="ExternalInput").ap()
    if FP8Z_PERIOD:
        ft8 = nc.dram_tensor("ft8", [128, 2, N_FUNC], F8,
                             kind="ExternalInput").ap()
        qt8 = nc.dram_tensor("qt8", [128, 2, QROWS], F8,
                             kind="ExternalInput").ap()
    out = nc.dram_tensor("out", [QROWS, D], f32, kind="ExternalOutput").ap()

    EXP = mybir.ActivationFunctionType.Exp
    COPY = mybir.ActivationFunctionType.Copy
    DR = mybir.MatmulPerfMode.DoubleRow

    # ft is loaded in pieces (along the key axis) so the V' pre-phase can
    # begin as soon as the first piece lands; small leading pieces start
    # the pipeline fast, striped across two DMA queues below.
    FT_W = [256, 256, 512, 512, 512] + [1024] * 6
    FT_OFF = [sum(FT_W[:i]) for i in range(len(FT_W))]
    NFP = len(FT_W)

    with tile.TileContext(nc) as tc:
        with (
            tc.tile_pool(name="singles", bufs=1) as singles,
            tc.tile_pool(name="epool", bufs=E0 + 2) as epool,
            tc.tile_pool(name="opool", bufs=4) as opool,
            tc.tile_pool(name="small", bufs=8) as small,
            tc.tile_pool(name="pz", bufs=2, space="PSUM") as pz,
            tc.tile_pool(name="pu", bufs=1, space="PSUM") as pu,
        ):
            wvt_sb = singles.tile([128, 2, D], BF)
            bias_sb = singles.tile([128, D], f32)
            v_sb = singles.tile([128, NKT, NV], F8)
            ebias_sb = singles.tile([128, 1], f32)
            if FP8Z_PERIOD:
                ft8_sb = singles.tile([128, 2, N_FUNC], F8)
                qt8_sb = singles.tile([128, 2, QROWS], F8)

            # warm-up matmuls on memset scratch (no DMA dependency): they
            # lift HAM to full clock so the first real tiles run at 2.4GHz.
            dw_t = singles.tile([128, 128], BF)
            dm_t = singles.tile([128, QB], BF)
            nc.vector.memset(dw_t, 0)
            nc.vector.memset(dm_t, 0)
            nc.vector.memset(ebias_sb, EBIAS)
            NDUMMY = 10
            pd_t = pz.tile([128, 2, QB], f32, tag="pz", name="pdummy")
            for i in range(NDUMMY):
                nc.tensor.matmul(pd_t[:, 0, :], lhsT=dw_t, rhs=dm_t,
                                 start=(i == 0), stop=(i == NDUMMY - 1))

            # ones column of V' (the softmax-denominator trick) is static
            nc.vector.memset(v_sb[:, :, D:D + 1], 1.0)

            ftp = [singles.tile([128, 2, FT_W[p]], BF, name=f"ftp{p}")
                   for p in range(NFP)]
            qtp = [singles.tile([128, 2, QB], BF, name=f"qtp{b}")
                   for b in range(NQB)]

            # input descriptors split over the two fast HWDGE paths (SP and
            # ACT) so the ft stream that gates the V' pre-phase drains at
            # double bandwidth. Within each queue: consumption order.
            def ft_dma(eng, p):
                eng.dma_start(out=ftp[p],
                              in_=ft[:, :, FT_OFF[p]:FT_OFF[p] + FT_W[p]])

            # stripe the ft pieces across both queues in consumption order so
            # piece p lands ~p/2 transfer-times in
            for p in range(1, NFP, 2):
                ft_dma(nc.scalar, p)
            if FP8Z_PERIOD:
                nc.scalar.dma_start(out=ft8_sb, in_=ft8)
            nc.sync.dma_start(out=wvt_sb, in_=wvt)
            ft_dma(nc.sync, 0)
            nc.sync.dma_start(out=qtp[0], in_=qt[:, :, 0:QB])
            for p in range(2, NFP, 2):
                ft_dma(nc.sync, p)
            nc.sync.dma_start(out=bias_sb, in_=bias)
            if FP8Z_PERIOD:
                nc.sync.dma_start(out=qt8_sb, in_=qt8)
            for b in range(1, NQB):
                nc.sync.dma_start(out=qtp[b], in_=qt[:, :, b * QB:(b + 1) * QB])

            def ft_sl(k, c):
                r = k * 128
                for p in range(NFP):
                    if r < FT_OFF[p] + FT_W[p]:
                        j = (r - FT_OFF[p]) // 128
                        return ftp[p][:, c, j * 128:(j + 1) * 128]
                raise AssertionError(k)

            def zexp(b, p, eq):
                """scores + exp for k-tile pair p of block b; exp result
                tile is appended to eq for the U matmuls to consume."""
                pz_t = pz.tile([128, 2, QB], f32, tag="pz", name=f"pz{b}_{p}")
                if True:
                    for h in (0, 1):
                        k = 2 * p + h
                        if _fp8z(b, p):
                            nc.tensor.matmul(
                                pz_t[:, h, :],
                                lhsT=ft8_sb[:, :, k * 128:(k + 1) * 128],
                                rhs=qt8_sb[:, :, b * QB:(b + 1) * QB],
                                start=True, stop=True, perf_mode=DR)
                        else:
                            nc.tensor.matmul(pz_t[:, h, :], lhsT=ft_sl(k, 0),
                                             rhs=qtp[b][:, 0, :],
                                             start=True, stop=False)
                            nc.tensor.matmul(pz_t[:, h, :], lhsT=ft_sl(k, 1),
                                             rhs=qtp[b][:, 1, :],
                                             start=False, stop=True)
                e2_t = epool.tile([128, 2, QB], F8, tag="e", name=f"e{b}_{p}")
                # e = exp(z/16) * 2^-5, fp8, both k-tiles in one instr
                nc.scalar.activation(e2_t, pz_t, EXP,
                                     scale=1.0 / 16.0, bias=ebias_sb)
                eq.append(e2_t)

            # ---- pre-phase: V' = [func_embed @ Wv_w.T | 1] -> fp8 ---------
            # Wv_b is NOT added here: softmax rows sum to 1, so the bias is
            # folded into the final normalize (out = U/s + b).  PSUM scratch
            # comes from the pu banks (U accumulation starts later); the
            # f32->fp8 casts alternate DVE/GPSIMD (ACT is busy with the
            # woven-in exps).  Block 0's first E0 score pairs + exps are
            # interleaved so the PE fills ft-DMA stalls and the flash loop
            # starts with a full e-queue.  The whole phase pipelines behind
            # the two-queue ft DMA stream.
            eq0 = []
            started0 = 0
            for j in range(NKT):
                pv = pu.tile([128, NV], f32, tag=f"pu{j % 4}", name=f"pv{j}")
                nc.tensor.matmul(pv[:, :D], lhsT=ft_sl(j, 0),
                                 rhs=wvt_sb[:, 0, :], start=True, stop=False)
                nc.tensor.matmul(pv[:, :D], lhsT=ft_sl(j, 1),
                                 rhs=wvt_sb[:, 1, :], start=False, stop=True)
                if j % 3 != 2:
                    nc.vector.tensor_copy(v_sb[:, j, :D], pv[:, :D])
                else:
                    nc.scalar.activation(v_sb[:, j, :D], pv[:, :D], COPY)
                if j % 6 == 5 and started0 < E0:
                    zexp(0, started0, eq0)
                    started0 += 1

            # ---- flash loop over query blocks ------------------------------
            for b in range(NQB):
                eq = eq0 if b == 0 else []
                started = started0 if b == 0 else 0
                pu_ts = [pu.tile([128, NV], f32, tag=f"pu{s}",
                                 name=f"pu{s}_{b}") for s in range(NSUB)]
                # z is emitted TWO pairs ahead of its U consumer: the
                # scheduler then starts each score pair the moment its PSUM
                # slot frees (the z->exp chain is the critical loop), with
                # U matmuls filling the remaining PE time.
                for p in range(NP):
                    while started < min(NP, p + 3):
                        zexp(b, started, eq)
                        started += 1
                    e2_t = eq.pop(0)
                    for s in range(NSUB):
                        nc.tensor.matmul(pu_ts[s],
                                         lhsT=e2_t[:, :, s * 128:(s + 1) * 128],
                                         rhs=v_sb[:, 2 * p:2 * p + 2, :],
                                         start=(p == 0), stop=(p == NP - 1),
                                         perf_mode=DR)

                for s in range(NSUB):
                    sr = small.tile([128, 1], f32)
                    nc.vector.reciprocal(sr, pu_ts[s][:, D:D + 1])
                    o_t = opool.tile([128, D], f32)
                    nc.vector.tensor_scalar_mul(o_t, pu_ts[s][:, :D], sr)
                    # bias add on GPSIMD (SBUF-only op): frees DVE and the
                    # pu bank sooner, shortening the block-tail chain
                    nc.gpsimd.tensor_add(o_t, o_t, bias_sb)
                    r0 = b * QB + s * 128
                    nc.sync.dma_start(out=out[r0:r0 + 128, :], in_=o_t)

    nc.compile()
    return nc


def _prep_inputs(func_embed, sent_embed, Wv_w, Wv_b):
    F = np.ascontiguousarray(np.asarray(func_embed, dtype=np.float32))
    Q = np.ascontiguousarray(np.asarray(sent_embed, dtype=np.float32))
    W = np.asarray(Wv_w, dtype=np.float32)
    b = np.asarray(Wv_b, dtype=np.float32)

    # device layout [p, c, n]: row p holds both 128-row d-chunks (c=0: d=p,
    # c=1: d=128+p) so each load is a single 3D-strided descriptor
    ft_f32 = np.ascontiguousarray(F.T.reshape(2, 128, N_FUNC).transpose(1, 0, 2))
    ft_h = ft_f32.astype(BF_NP)
    wvt_h = np.ascontiguousarray(
        W.T.reshape(2, 128, D).transpose(1, 0, 2)).astype(BF_NP)
    bias_h = np.ascontiguousarray(np.broadcast_to(b, (128, D))).astype(np.float32)
    if FP8Z_PERIOD:
        ft8_h = ft_f32.astype(F8_NP)

    in_maps = []
    for i in range(N_CORES):
        qs = Q[i * QROWS:(i + 1) * QROWS]
        qt_f32 = np.ascontiguousarray(
            qs.T.reshape(2, 128, QROWS).transpose(1, 0, 2))
        m = {"ft": ft_h, "qt": qt_f32.astype(BF_NP), "wvt": wvt_h,
             "bias": bias_h}
        if FP8Z_PERIOD:
            m["ft8"] = ft8_h
            m["qt8"] = qt_f32.astype(F8_NP)
        in_maps.append(m)
    return in_maps


def run(inputs, trace=False, **kw):
    global _COMPILED
    if _COMPILED is None:
        _COMPILED = build()
    in_maps = _prep_inputs(**inputs)
    res = run_bass_kernel_spmd(_COMPILED, in_maps, list(range(N_CORES)),
                               trace=trace, **kw)
    out = np.concatenate([res.results[i]["out"] for i in range(N_CORES)], axis=0)
    return out, res


def kernel(**inputs):
    out, _ = run(inputs, trace=False)
    return out
